# revision 7
# baseline (speedup 1.0000x reference)
"""Trainium2 Bass kernel for nn_Decoder (GRU decoder + vocab projection + argmax).

Strategy (8 NeuronCores):
- All matmuls in fp16 hi/lo 3-pass split (measured ~22 effective mantissa bits,
  i.e. fp32-grade) so logits/argmax match the fp32 reference.
- GRU hidden dim H=1024 is sharded 8 ways (128 rows/core). Each step computes
  the local gate rows, then an AllGather exchanges the 128-row h shard so every
  core has the full h for the next step's contraction.
- Output projection is vocab-sharded (3750 rows/core, padded to 3840), computed
  from SBUF-resident hidden states; per-core argmax via max/max_index; host
  combines the 8 shard argmaxes.
- Embedding lookup on device via indirect DMA gather + PE transpose; the x-side
  gates for all timesteps are precomputed in one sharded matmul.
"""
import sys

sys.path.insert(0, "/opt/trn_rl_repo")

import numpy as np
import ml_dtypes

import concourse.bass as bass
import concourse.mybir as mybir
import concourse.tile as tile
from concourse import bacc
from concourse.bass import ts, ds
from concourse.bass_utils import run_bass_kernel_spmd
from concourse.masks import make_identity

P = 128
B = 64
T = 50
H = 1024
KH = 8          # H / 128 contraction chunks
E = 300
KE = 3          # ceil(E/128) chunks (E padded to 384)
NPOS = B * T    # 3200 positions, p = t*64 + b
MPOS = NPOS // P  # 25 position tiles
V = 30000
VL = V // 8     # 3750 real vocab rows per core
VLP = 3840      # padded to 30*128
NV = 8          # vocab n-tiles per core: 7x512 + 1x256
NCORE = 8

F16 = mybir.dt.float16
F32 = mybir.dt.float32
I32 = mybir.dt.int32
U32 = mybir.dt.uint32

_CACHE = {}


def _split16(x):
    hi = x.astype(np.float16)
    lo = (x.astype(np.float32) - hi.astype(np.float32)).astype(np.float16)
    return hi, lo


def _build():
    nc = bacc.Bacc()

    def din(name, shape, dt):
        return nc.declare_dram_parameter(name, list(shape), dt, isOutput=False)

    def dout(name, shape, dt):
        return nc.declare_dram_parameter(name, list(shape), dt, isOutput=True)

    idx_d = din("idx", [P, MPOS], I32)
    emb_d = din("emb", [V, E], F32)
    wih_hi_d = din("wih_hi", [P, KE, 3, P], F16)
    wih_lo_d = din("wih_lo", [P, KE, 3, P], F16)
    bxg_d = din("bxg", [P, 3], F32)
    bnh_d = din("bnh", [P, 1], F32)
    whh_hi_d = din("whh_hi", [P, KH, 3, P], F16)
    whh_lo_d = din("whh_lo", [P, KH, 3, P], F16)
    h0s_d = din("h0s", [P, B], F32)
    h0hi_d = din("h0hi", [P, KH, B], F16)
    h0lo_d = din("h0lo", [P, KH, B], F16)
    wout_hi_d = din("wout_hi", [P, KH, VLP], F16)
    wout_lo_d = din("wout_lo", [P, KH, VLP], F16)
    bo_d = din("bo", [P, VLP], F32)
    iota8_d = din("iota8", [P, NV], F32)

    logits_d = dout("logits_part", [NPOS, VL], F32)
    hT_d = dout("h_part", [T, P, B], F32)
    maxv_d = dout("maxv", [P, MPOS], F32)
    maxi_d = dout("maxi", [P, MPOS], F32)

    cc_in = nc.dram_tensor("cc_in", [P, 2, B], F16)
    cc_out = nc.dram_tensor("cc_out", [NCORE, P, 2, B], F16, addr_space="Shared")

    AG = mybir.AluOpType
    TANH = mybir.ActivationFunctionType.Tanh
    COPY = mybir.ActivationFunctionType.Copy

    from contextlib import ExitStack

    with ExitStack() as ctx:
        tc = ctx.enter_context(tile.TileContext(nc))
        s_do = ctx.enter_context(nc.semaphore())
        s_cc = ctx.enter_context(nc.semaphore())
        s_di = ctx.enter_context(nc.semaphore())

        cpool = ctx.enter_context(tc.tile_pool(name="const", bufs=1))

        idx_sb = cpool.tile([P, MPOS], I32)
        nc.sync.dma_start(idx_sb[:], idx_d[:])
        wih_hi = cpool.tile([P, KE, 3, P], F16)
        nc.sync.dma_start(wih_hi[:], wih_hi_d[:])
        wih_lo = cpool.tile([P, KE, 3, P], F16)
        nc.sync.dma_start(wih_lo[:], wih_lo_d[:])
        bxg = cpool.tile([P, 3], F32)
        nc.sync.dma_start(bxg[:], bxg_d[:])
        bnh = cpool.tile([P, 1], F32)
        nc.sync.dma_start(bnh[:], bnh_d[:])
        whh_hi = cpool.tile([P, KH, 3, P], F16)
        nc.sync.dma_start(whh_hi[:], whh_hi_d[:])
        whh_lo = cpool.tile([P, KH, 3, P], F16)
        nc.sync.dma_start(whh_lo[:], whh_lo_d[:])
        h0s = cpool.tile([P, B], F32)
        nc.sync.dma_start(h0s[:], h0s_d[:])
        h0hi = cpool.tile([P, KH, B], F16)
        nc.sync.dma_start(h0hi[:], h0hi_d[:])
        h0lo = cpool.tile([P, KH, B], F16)
        nc.sync.dma_start(h0lo[:], h0lo_d[:])
        iota8 = cpool.tile([P, NV], F32)
        nc.sync.dma_start(iota8[:], iota8_d[:])
        ident = cpool.tile([P, P], F32)
        make_identity(nc, ident[:])

        hsT_hi = cpool.tile([P, KH, NPOS], F16)     # full h per step (post-gather)
        hsT_lo = cpool.tile([P, KH, NPOS], F16)

        xg_stack = ExitStack()
        xgp = xg_stack.enter_context(tc.tile_pool(name="xgp", bufs=1))
        xg = xgp.tile([P, 3, NPOS], F32)            # x-side gates, phases 1-2 only

        # ---------------- Phase 1: gather + transpose + x-gates ----------------
        # Processed in blocks of 512 positions to bound SBUF: gather 4 position
        # tiles, PE-transpose to embT layout with fp16 hi/lo split, then the
        # 3-pass x-gate matmuls for this block.
        with (
            tc.tile_pool(name="p1", bufs=3) as p1,
            tc.tile_pool(name="p1e", bufs=2) as p1e,
            tc.tile_pool(name="p1ps", bufs=4, space="PSUM") as p1ps,
        ):
            for nb in range(7):
                nw = 512 if nb < 6 else NPOS - 6 * 512  # 128
                mts = nw // P  # position tiles in this block
                ehi = p1e.tile([P, KE, 512], F16, tag="ehi")
                elo = p1e.tile([P, KE, 512], F16, tag="elo")
                nc.vector.memzero(ehi[:, 2, :])
                nc.vector.memzero(elo[:, 2, :])
                for mi in range(mts):
                    m = nb * 4 + mi
                    gx = p1.tile([P, E], F32, tag="gx")
                    nc.gpsimd.indirect_dma_start(
                        out=gx[:],
                        out_offset=None,
                        in_=emb_d[:],
                        in_offset=bass.IndirectOffsetOnAxis(
                            ap=idx_sb[:, m : m + 1], axis=0
                        ),
                    )
                    for e in range(KE):
                        w = P if e < 2 else E - 2 * P  # 128,128,44
                        pt = p1ps.tile([P, P], F32, tag="tp")
                        nc.tensor.transpose(
                            pt[:w, :], gx[:, e * P : e * P + w], ident[:]
                        )
                        nc.vector.tensor_copy(ehi[:w, e, ts(mi, P)], pt[:w, :])
                        lo32 = p1.tile([P, P], F32, tag="lo32")
                        nc.vector.tensor_copy(lo32[:w, :], ehi[:w, e, ts(mi, P)])
                        nc.vector.tensor_tensor(
                            elo[:w, e, ts(mi, P)], pt[:w, :], lo32[:w, :],
                            op=AG.subtract,
                        )

                for m3 in range(3):
                    pxg = p1ps.tile([P, 512], F32, tag="xps")
                    for e in range(KE):
                        nc.tensor.matmul(
                            pxg[:, :nw], wih_hi[:, e, m3, :], ehi[:, e, :nw],
                            start=(e == 0), stop=False,
                        )
                        nc.tensor.matmul(
                            pxg[:, :nw], wih_hi[:, e, m3, :], elo[:, e, :nw],
                            start=False, stop=False,
                        )
                        nc.tensor.matmul(
                            pxg[:, :nw], wih_lo[:, e, m3, :], ehi[:, e, :nw],
                            start=False, stop=(e == KE - 1),
                        )
                    nc.vector.tensor_scalar_add(
                        xg[:, m3, ds(nb * 512, nw)], pxg[:, :nw], bxg[:, m3 : m3 + 1]
                    )

        # ---------------- Phase 2: GRU recurrence, H-sharded ----------------
        with (
            tc.tile_pool(name="p2", bufs=3) as p2,
            tc.tile_pool(name="p2ps", bufs=2, space="PSUM") as p2ps,
        ):
            prev_h = h0s
            for t in range(T):
                if t == 0:
                    rhs_hi, rhs_lo, off = h0hi, h0lo, 0
                else:
                    rhs_hi, rhs_lo, off = hsT_hi, hsT_lo, (t - 1) * B

                ps = p2ps.tile([P, 3, B], F32, tag="g")
                for m3 in range(3):
                    for k in range(KH):
                        rh = rhs_hi[:, k, ds(off, B)]
                        rl = rhs_lo[:, k, ds(off, B)]
                        nc.tensor.matmul(ps[:, m3, :], whh_hi[:, k, m3, :], rh,
                                         start=(k == 0), stop=False)
                        nc.tensor.matmul(ps[:, m3, :], whh_lo[:, k, m3, :], rh,
                                         start=False, stop=False)
                        nc.tensor.matmul(ps[:, m3, :], whh_hi[:, k, m3, :], rl,
                                         start=False, stop=(k == KH - 1))

                pre = p2.tile([P, 2, B], F32, tag="pre")
                nc.vector.tensor_tensor(pre[:], ps[:, 0:2, :], xg[:, 0:2, ts(t, B)],
                                        op=AG.add)
                trz = p2.tile([P, 2, B], F32, tag="trz")
                nc.scalar.activation(trz[:], pre[:], TANH, scale=0.5)
                # hb2 = 0.5*(h_gate_n) + 0.5*b_hh_n   (bnh staged pre-halved)
                hb2 = p2.tile([P, B], F32, tag="hb2")
                nc.vector.tensor_scalar(hb2[:], ps[:, 2, :], 0.5, bnh[:, 0:1],
                                        op0=AG.mult, op1=AG.add)
                u = p2.tile([P, B], F32, tag="u")
                nc.vector.tensor_tensor(u[:], trz[:, 0, :], hb2[:], op=AG.mult)
                v = p2.tile([P, B], F32, tag="v")
                nc.vector.tensor_tensor(v[:], hb2[:], xg[:, 2, ts(t, B)], op=AG.add)
                narg = p2.tile([P, B], F32, tag="narg")
                nc.vector.tensor_tensor(narg[:], u[:], v[:], op=AG.add)
                tn = p2.tile([P, B], F32, tag="tn")
                nc.scalar.activation(tn[:], narg[:], TANH)
                aa = p2.tile([P, B], F32, tag="aa")
                nc.vector.tensor_tensor(aa[:], prev_h[:], tn[:], op=AG.subtract)
                cc_t = p2.tile([P, B], F32, tag="cc")
                nc.vector.tensor_tensor(cc_t[:], trz[:, 1, :], aa[:], op=AG.mult)
                dd = p2.tile([P, B], F32, tag="dd")
                nc.vector.tensor_tensor(dd[:], prev_h[:], tn[:], op=AG.add)
                ee = p2.tile([P, B], F32, tag="ee")
                nc.vector.tensor_tensor(ee[:], cc_t[:], dd[:], op=AG.add)
                h_new = p2.tile([P, B], F32, tag="hnew")
                nc.scalar.activation(h_new[:], ee[:], COPY, scale=0.5)

                hl = p2.tile([P, 2, B], F16, tag="hl")
                nc.vector.tensor_copy(hl[:, 0, :], h_new[:])
                hback = p2.tile([P, B], F32, tag="hback")
                nc.vector.tensor_copy(hback[:], hl[:, 0, :])
                nc.vector.tensor_tensor(hl[:, 1, :], h_new[:], hback[:],
                                        op=AG.subtract)

                nc.sync.dma_start(hT_d[t], h_new[:])

                with tc.tile_critical():
                    d0 = nc.gpsimd.dma_start(cc_in[:], hl[:])
                    if t > 0:
                        # previous step's gathers must finish before cc rewrites
                        # cc_out; the wait blocks the gpsimd queue ahead of cc.
                        d0._wait_ge(s_di, 32 * t)
                    d0.then_inc(s_do, 16)
                    cc = nc.gpsimd.collective_compute(
                        "AllGather",
                        AG.bypass,
                        replica_groups=[list(range(NCORE))],
                        ins=[cc_in[:]],
                        outs=[cc_out[:]],
                    )
                    cc._wait_ge(s_do, 16 * (t + 1))
                    cc.then_inc(s_cc, 1)
                    d1 = nc.gpsimd.dma_start(
                        hsT_hi[:, :, ts(t, B)],
                        cc_out.ap()[:, :, 0, :].rearrange("k p b -> p k b"),
                    )
                    d1._wait_ge(s_cc, t + 1)
                    d1.then_inc(s_di, 16)
                    d2 = nc.gpsimd.dma_start(
                        hsT_lo[:, :, ts(t, B)],
                        cc_out.ap()[:, :, 1, :].rearrange("k p b -> p k b"),
                    )
                    d2.then_inc(s_di, 16)

                prev_h = h_new

        xg_stack.close()

        # ---------------- Phase 3: vocab-sharded projection + argmax ----------------
        with (
            tc.tile_pool(name="p3w", bufs=2) as p3w,
            tc.tile_pool(name="p3b", bufs=1) as p3b,
            tc.tile_pool(name="p3s", bufs=3) as p3s,
            tc.tile_pool(name="p3sm", bufs=4) as p3sm,
            tc.tile_pool(name="p3ps", bufs=4, space="PSUM") as p3ps,
        ):
            bo_sb = p3b.tile([P, VLP], F32)
            nc.sync.dma_start(bo_sb[:], bo_d[:])
            cand_v = p3b.tile([P, MPOS, NV], F32)
            cand_i = p3b.tile([P, MPOS, NV], F32)
            for n in range(NV):
                nw = 512 if n < NV - 1 else VLP - 512 * (NV - 1)  # 512 / 256
                wh = p3w.tile([P, KH, 512], F16, tag="wh")
                nc.sync.dma_start(wh[:, :, :nw], wout_hi_d.ap()[:, :, ds(n * 512, nw)])
                wl = p3w.tile([P, KH, 512], F16, tag="wl")
                nc.sync.dma_start(wl[:, :, :nw], wout_lo_d.ap()[:, :, ds(n * 512, nw)])
                for m in range(MPOS):
                    pp = p3ps.tile([P, 512], F32, tag="pps")
                    for k in range(KH):
                        hh = hsT_hi[:, k, ts(m, P)]
                        hlo = hsT_lo[:, k, ts(m, P)]
                        nc.tensor.matmul(pp[:, :nw], hh, wh[:, k, :nw],
                                         start=(k == 0), stop=False)
                        nc.tensor.matmul(pp[:, :nw], hh, wl[:, k, :nw],
                                         start=False, stop=False)
                        nc.tensor.matmul(pp[:, :nw], hlo, wh[:, k, :nw],
                                         start=False, stop=(k == KH - 1))
                    st = p3s.tile([P, 512], F32, tag="st")
                    nc.vector.tensor_tensor(st[:, :nw], pp[:, :nw],
                                            bo_sb[:, ds(n * 512, nw)], op=AG.add)
                    lo_col = n * 512
                    hi_col = min(VL, n * 512 + nw)
                    if hi_col > lo_col:
                        rw = hi_col - lo_col
                        nc.sync.dma_start(
                            logits_d.ap()[ts(m, P), ds(lo_col, rw)], st[:, :rw]
                        )
                    v8 = p3sm.tile([P, 8], F32, tag="v8")
                    nc.vector.max(v8[:], st[:, :nw])
                    i8 = p3sm.tile([P, 8], U32, tag="i8")
                    nc.vector.max_index(i8[:], v8[:], st[:, :nw])
                    nc.vector.tensor_copy(cand_v[:, m, n : n + 1], v8[:, 0:1])
                    gi = p3sm.tile([P, 1], F32, tag="gi")
                    nc.vector.tensor_copy(gi[:], i8[:, 0:1])
                    nc.vector.tensor_scalar_add(cand_i[:, m, n : n + 1], gi[:],
                                                float(n * 512))

            maxv_sb = p3s.tile([P, MPOS], F32, tag="mxv")
            maxi_sb = p3s.tile([P, MPOS], F32, tag="mxi")
            for m in range(MPOS):
                w8v = p3sm.tile([P, 8], F32, tag="w8v")
                nc.vector.max(w8v[:], cand_v[:, m, :])
                w8i = p3sm.tile([P, 8], U32, tag="w8i")
                nc.vector.max_index(w8i[:], w8v[:], cand_v[:, m, :])
                wj = p3sm.tile([P, 1], F32, tag="wj")
                nc.vector.tensor_copy(wj[:], w8i[:, 0:1])
                mask = p3sm.tile([P, NV], F32, tag="mask")
                nc.vector.tensor_scalar(mask[:], iota8[:], wj[:, 0:1], None,
                                        op0=AG.is_equal)
                prod = p3sm.tile([P, NV], F32, tag="prod")
                nc.vector.tensor_tensor(prod[:], mask[:], cand_i[:, m, :], op=AG.mult)
                nc.vector.tensor_reduce(maxi_sb[:, m : m + 1], prod[:],
                                        axis=mybir.AxisListType.X, op=AG.add)
                nc.vector.tensor_copy(maxv_sb[:, m : m + 1], w8v[:, 0:1])
            nc.sync.dma_start(maxv_d[:], maxv_sb[:])
            nc.sync.dma_start(maxi_d[:], maxi_sb[:])

    nc.finalize()
    return nc


def kernel(inputs, max_length, init_hidden, att_embedding, embedding,
           W_ih, W_hh, b_ih, b_hh, W_out, b_out):
    inputs = np.asarray(inputs)
    init_hidden = np.asarray(init_hidden, dtype=np.float32)
    att_embedding = np.asarray(att_embedding, dtype=np.float32)
    embedding = np.asarray(embedding, dtype=np.float32)
    W_ih = np.asarray(W_ih, dtype=np.float32)
    W_hh = np.asarray(W_hh, dtype=np.float32)
    b_ih = np.asarray(b_ih, dtype=np.float32)
    b_hh = np.asarray(b_hh, dtype=np.float32)
    W_out = np.asarray(W_out, dtype=np.float32)
    b_out = np.asarray(b_out, dtype=np.float32)
    assert int(max_length) == T

    if "nc" not in _CACHE:
        _CACHE["nc"] = _build()
    nc = _CACHE["nc"]

    # ---- host staging ----
    idx = np.zeros((P, MPOS), np.int32)
    pg = np.arange(NPOS)
    tt_, bb_ = pg // B, pg % B
    idx[pg % P, pg // P] = inputs[bb_, tt_].astype(np.int32)

    # W_ihT padded to 384 rows
    wihT = np.zeros((KE * P, 3 * H), np.float32)
    wihT[:E] = W_ih.T
    whhT = W_hh.T  # [H, 3H]

    h0 = np.concatenate([init_hidden[0], att_embedding], axis=1)  # [B, H]
    h0T = h0.T  # [H, B]
    h0T_hi, h0T_lo = _split16(h0T)

    bxg_full = b_ih + np.concatenate([b_hh[:H], b_hh[H:2*H], np.zeros(H, np.float32)])

    iota8 = np.tile(np.arange(NV, dtype=np.float32), (P, 1))

    in_maps = []
    for c in range(NCORE):
        rows = np.concatenate([
            np.arange(c * P, (c + 1) * P),
            H + np.arange(c * P, (c + 1) * P),
            2 * H + np.arange(c * P, (c + 1) * P),
        ])  # r,z,n shard rows in 3H
        wih_s = wihT[:, rows].reshape(KE, P, 3, P).transpose(1, 0, 2, 3)
        wih_hi, wih_lo = _split16(np.ascontiguousarray(wih_s))
        whh_s = whhT[:, rows].reshape(KH, P, 3, P).transpose(1, 0, 2, 3)
        whh_hi, whh_lo = _split16(np.ascontiguousarray(whh_s))
        bxg = bxg_full[rows].reshape(3, P).T.copy()  # [P, 3]
        bnh = (0.5 * b_hh[2 * H + c * P: 2 * H + (c + 1) * P]).reshape(P, 1).copy()

        wout_s = np.zeros((VLP, H), np.float32)
        wout_s[:VL] = W_out[c * VL:(c + 1) * VL]
        woutT = wout_s.T.reshape(KH, P, VLP).transpose(1, 0, 2)  # [P, KH, VLP]
        wout_hi, wout_lo = _split16(np.ascontiguousarray(woutT))
        bo = np.full(VLP, -1e9, np.float32)
        bo[:VL] = b_out[c * VL:(c + 1) * VL]
        bo = np.tile(bo, (P, 1))

        h0s = h0T[c * P:(c + 1) * P].copy()
        h0hi = h0T_hi.reshape(KH, P, B).transpose(1, 0, 2).copy()
        h0lo = h0T_lo.reshape(KH, P, B).transpose(1, 0, 2).copy()

        in_maps.append(dict(
            idx=idx, emb=embedding,
            wih_hi=wih_hi, wih_lo=wih_lo, bxg=bxg, bnh=bnh,
            whh_hi=whh_hi, whh_lo=whh_lo,
            h0s=h0s, h0hi=h0hi, h0lo=h0lo,
            wout_hi=wout_hi, wout_lo=wout_lo, bo=bo, iota8=iota8,
        ))

    res = run_bass_kernel_spmd(nc, in_maps, list(range(NCORE)))
    rs = res.results

    # ---- host assembly ----
    hiddens = np.zeros((B, T, H), np.float32)
    for c in range(NCORE):
        hp = rs[c]["h_part"]  # [T, P, B]
        hiddens[:, :, c * P:(c + 1) * P] = hp.transpose(2, 0, 1)

    logits_tm = np.concatenate([rs[c]["logits_part"] for c in range(NCORE)], axis=1)
    # positions are t-major (p = t*64+b); reference rows are b*T + t
    logits = logits_tm.reshape(T, B, V).transpose(1, 0, 2).reshape(B * T, V)

    vals = np.stack([rs[c]["maxv"].T.reshape(NPOS) for c in range(NCORE)])  # [8, NPOS]
    idxs = np.stack([rs[c]["maxi"].T.reshape(NPOS) for c in range(NCORE)])
    win = np.argmax(vals, axis=0)
    gidx = (idxs[win, np.arange(NPOS)] + win * VL).astype(np.int32)
    sampled = gidx.reshape(T, B).T.copy()  # [B, T]

    return hiddens, logits, sampled
